# revision 4
# baseline (speedup 1.0000x reference)
"""Trainium2 Bass kernel for the DIFSR 3-stream attention block (v5).

Reference math (B=32, L=512, H=512, NH=8, HD=64):
    V     = heads(V_id_input @ Wv.T)                        # biases are all zero
    total = sum_s heads(x_s @ Wq_s.T) @ heads(x_s @ Wk_s.T).T * HD**-0.5
            for s in (id, cate, brand)
    total += relative_time;  causal mask;  softmax over k
    out   = (softmax @ V).merge_heads() @ Wo.T

Sharding: pure data-parallel over batch B across the 8 NeuronCores.

v5 (vs v4): exploits measured HW tile-position concurrency (col/row-paired
matmuls overlap on the PE array) and software-pipelines across batches so the
PE never waits on the DVE/ACT softmax chain.

  - id+cate Q/K projected into per-head concat tiles [64 id | 64 cate, L] via
    col-paired M=64 matmuls (concurrent on HW); scores for both streams come
    from ONE K=128 matmul per head. Brand scores: K=64 row-paired pairs.
  - Scores are computed TRANSPOSED sT[k, q] with causal block skipping; the
    per-(pair,head) masked rel bias is pre-packed on the host into the same
    q-concat layout [128, 1280] (fp8, additive -240 mask) and ADDED ON THE PE
    as an fp8 identity matmul appended to each score accumulation chain --
    the DVE does no softmax adds at all; exp (ACT) reads the psum directly.
  - attn@V redesigned: per-pair V tiles [V_h0 | V_h1] with col-paired matmuls
    (eT_h0 / eT_h1 as the two streams) accumulate attnT for BOTH heads into
    one psum tile; a ones-pair tile accumulates both denominators likewise.
    Normalization is ONE full-width reciprocal + ONE multiply per pair.
  - Emission order pipelines batches: scores(b) -> projections(b+1) ->
    attn(b) -> out-proj(b), so DVE/ACT softmax work for batch b runs under
    the projection matmuls of batch b+1.
  - Startup: brand weights + brand x stream DMA'd first so the PE starts
    ~3us in; remaining weights stream in under compute.
"""

import sys

if "/opt/trn_rl_repo" not in sys.path:
    sys.path.insert(0, "/opt/trn_rl_repo")

import numpy as np

B, L, H, NH = 32, 512, 512, 8
HD = H // NH  # 64
NCORES = 8
BL = B // NCORES  # 4 batches per core
SCALE = HD**-0.5
P = 128
NT = L // P  # 4 k/q tiles
KC = H // P  # 4 contraction chunks
NPAIR = NH // 2  # 4 head pairs
MASK_VAL = -240.0  # representable in fp8_e4m3; exp(s-240) == 0 in f32

# causal q-concat layout: for k-tile j, q ranges over [j*P, L)
J_OFF = [0, 512, 896, 1152]
J_NQ = [512, 384, 256, 128]
QTOT = 1280

_built_nc = None


def build_nc(iters=1):
    import concourse.mybir as mybir
    from concourse import bacc
    from concourse.tile import TileContext

    f32 = mybir.dt.float32
    bf16 = mybir.dt.bfloat16
    fp8 = mybir.dt.float8e4
    Exp = mybir.ActivationFunctionType.Exp

    nc = bacc.Bacc("TRN2", target_bir_lowering=False, debug=False)

    # host-prearranged inputs (see make_host_inputs for layouts)
    # xT stream order: 0=brand, 1=id, 2=cate, 3=v
    xT = nc.dram_tensor("xT", [BL, P, 4, KC, L], bf16, kind="ExternalInput").ap()
    relc = nc.dram_tensor("relc", [BL, NPAIR, P, 2, QTOT], fp8, kind="ExternalInput").ap()
    w_icq = nc.dram_tensor("w_icq", [P, KC, NH, 2, HD], bf16, kind="ExternalInput").ap()
    w_ick = nc.dram_tensor("w_ick", [P, KC, NH, 2, HD], bf16, kind="ExternalInput").ap()
    w_bq = nc.dram_tensor("w_bq", [P, KC, H], bf16, kind="ExternalInput").ap()
    w_bk = nc.dram_tensor("w_bk", [P, KC, H], bf16, kind="ExternalInput").ap()
    w_v = nc.dram_tensor("w_v", [P, KC, H], bf16, kind="ExternalInput").ap()
    w_o = nc.dram_tensor("w_o", [P, KC, H], bf16, kind="ExternalInput").ap()
    identt = nc.dram_tensor("identt", [P, P], fp8, kind="ExternalInput").ap()
    out = nc.dram_tensor("out", [BL, P, NT, H], bf16, kind="ExternalOutput").ap()

    with TileContext(nc) as tc:
        with (
            tc.tile_pool(name="wsb", bufs=1) as wsb,
            tc.tile_pool(name="xt", bufs=2) as xtp,
            tc.tile_pool(name="rel", bufs=2) as relp,
            tc.tile_pool(name="qk", bufs=2) as qkp,
            tc.tile_pool(name="vp", bufs=2) as vpp,
            tc.tile_pool(name="ssb", bufs=2) as ssp,
            tc.tile_pool(name="et", bufs=2) as etp,
            tc.tile_pool(name="att", bufs=2) as attp,
            tc.tile_pool(name="yout", bufs=1) as youtp,
            tc.tile_pool(name="ppsum", bufs=3, space="PSUM") as ppsum,
            tc.tile_pool(name="spsum", bufs=3, space="PSUM") as spsum,
            tc.tile_pool(name="apsum", bufs=2, space="PSUM") as apsum,
        ):
            # ---- weights, brand first so the PE can start early ----
            Wbq = wsb.tile([P, KC, H], bf16, name="Wbq")
            nc.sync.dma_start(out=Wbq, in_=w_bq)
            Wbk = wsb.tile([P, KC, H], bf16, name="Wbk")
            nc.sync.dma_start(out=Wbk, in_=w_bk)

            xt0 = xtp.tile([P, 4, KC, L], bf16, name="xt_0", tag="xt")
            nc.sync.dma_start(out=xt0[:, 0], in_=xT[0, :, 0])  # brand stream

            Wicq = wsb.tile([P, KC, NH, 2, HD], bf16, name="Wicq")
            nc.sync.dma_start(out=Wicq, in_=w_icq)
            Wick = wsb.tile([P, KC, NH, 2, HD], bf16, name="Wick")
            nc.sync.dma_start(out=Wick, in_=w_ick)
            Ident = wsb.tile([P, P], fp8, name="Ident")
            nc.sync.dma_start(out=Ident, in_=identt)
            nc.sync.dma_start(out=xt0[:, 1:3], in_=xT[0, :, 1:3])  # id+cate
            nc.sync.dma_start(out=xt0[:, 3], in_=xT[0, :, 3])  # v
            Wv = wsb.tile([P, KC, H], bf16, name="Wv")
            nc.sync.dma_start(out=Wv, in_=w_v)
            Wo = wsb.tile([P, KC, H], bf16, name="Wo")
            nc.sync.dma_start(out=Wo, in_=w_o)

            # ones pair tile for the denominator matmuls
            onesp = wsb.tile([P, P], bf16, name="onesp")
            nc.gpsimd.memset(onesp, 1.0)

            # PSUM->SBUF drains alternate engines; DVE carries adds/recip/mul
            rr = [0]

            def cpy(dst, src):
                rr[0] += 1
                if rr[0] % 3 == 0:
                    nc.scalar.copy(dst, src)
                else:
                    nc.vector.tensor_copy(dst, src)

            def dma_in(b):
                """Prefetch xt(b) and rel(b); returns (xt, rels)."""
                if b == 0:
                    xt = xt0
                else:
                    xt = xtp.tile([P, 4, KC, L], bf16, name=f"xt_{b}", tag="xt")
                    nc.sync.dma_start(out=xt, in_=xT[b])
                rels = []
                for c in range(NPAIR):
                    rl = relp.tile([P, 2, QTOT], fp8, name=f"rel_{c}_{b}", tag=f"rel{c}")
                    nc.sync.dma_start(out=rl, in_=relc[b, c])
                    rels.append(rl)
                return xt, rels

            def project(b, xt):
                """All projections for batch b -> (Qic, Kic, Qb, Kb, Vp)."""
                # brand Q/K: head-pair chunk tiles [h2c | h2c+1, L]
                def project_b(wsb_t, kind):
                    tiles = []
                    for c in range(NPAIR):
                        pp = ppsum.tile([P, L], f32, name=f"pp{kind}_{c}_{b}", tag="pp")
                        for kc in range(KC):
                            nc.tensor.matmul(
                                pp,
                                wsb_t[:, kc, c * P : (c + 1) * P],
                                xt[:, 0, kc, :],
                                start=(kc == 0),
                                stop=(kc == KC - 1),
                            )
                        t = qkp.tile([P, L], bf16, name=f"{kind}_{c}_{b}", tag=f"{kind}_{c}")
                        cpy(t, pp)
                        tiles.append(t)
                    return tiles

                Qb = project_b(Wbq, "Qb")
                Kb = project_b(Wbk, "Kb")

                # id+cate Q/K: per-head concat tiles [64 id | 64 cate, L]
                # via col-paired M=64 matmuls (concurrent on HW)
                def project_ic(wsb_t, kind):
                    tiles = []
                    for h in range(NH):
                        pp = ppsum.tile([P, L], f32, name=f"pp{kind}_{h}_{b}", tag="pp")
                        for kc in range(KC):
                            nc.tensor.matmul(
                                pp[0:HD, :],
                                wsb_t[:, kc, h, 0, :],
                                xt[:, 1, kc, :],
                                start=(kc == 0),
                                stop=(kc == KC - 1),
                            )
                            nc.tensor.matmul(
                                pp[HD:P, :],
                                wsb_t[:, kc, h, 1, :],
                                xt[:, 2, kc, :],
                                start=(kc == 0),
                                stop=(kc == KC - 1),
                            )
                        t = qkp.tile([P, L], bf16, name=f"{kind}_{h}_{b}", tag=f"{kind}_{h}")
                        cpy(t, pp)
                        tiles.append(t)
                    return tiles

                Qic = project_ic(Wicq, "Qic")
                Kic = project_ic(Wick, "Kic")

                # V into per-pair tiles: Vp[j][:, c*128:(c+1)*128] = [V_h0 | V_h1]
                Vp = []
                for j in range(NT):
                    pp = ppsum.tile([P, H], f32, name=f"ppv_{j}_{b}", tag="pp")
                    for kc in range(KC):
                        nc.tensor.matmul(
                            pp,
                            xt[:, 3, kc, j * P : (j + 1) * P],
                            Wv[:, kc, :],
                            start=(kc == 0),
                            stop=(kc == KC - 1),
                        )
                    t = vpp.tile([P, H], bf16, name=f"Vp_{j}_{b}", tag=f"Vp_{j}")
                    cpy(t, pp)
                    Vp.append(t)
                return Qic, Kic, Qb, Kb, Vp

            def scores(b, Qic, Kic, Qb, Kb, rels):
                """Masked scores (+rel via fp8 identity matmul) -> exp, as eT
                concat tiles [128, QTOT] bf16."""
                eT = {}
                for c in range(NPAIR):
                    rl = rels[c]
                    for hh in range(2):
                        h = 2 * c + hh
                        hsl = slice(hh * HD, (hh + 1) * HD)
                        e = etp.tile([P, QTOT], bf16, name=f"eT_{h}_{b}", tag=f"eT_{h}")
                        for grp, js in (("j0", (0,)), ("j1", (1,)), ("j23", (2, 3))):
                            w = sum(J_NQ[j] for j in js)
                            o0 = J_OFF[js[0]]
                            t = spsum.tile([P, w], f32, name=f"sp{grp}_{h}_{b}", tag="sp")
                            off = 0
                            for j in js:
                                nq = J_NQ[j]
                                jsl = slice(j * P, (j + 1) * P)
                                qsl = slice(j * P, L)
                                nc.tensor.matmul(
                                    t[:, off : off + nq],
                                    Kic[h][:, jsl],
                                    Qic[h][:, qsl],
                                    start=True,
                                    stop=False,
                                )
                                nc.tensor.matmul(
                                    t[:, off : off + nq],
                                    Kb[c][hsl, jsl],
                                    Qb[c][hsl, qsl],
                                    start=False,
                                    stop=False,
                                )
                                nc.tensor.matmul(
                                    t[:, off : off + nq],
                                    Ident,
                                    rl[:, hh, J_OFF[j] : J_OFF[j] + nq],
                                    start=False,
                                    stop=True,
                                )
                                off += nq
                            nc.scalar.activation(e[:, o0 : o0 + w], t, Exp)
                        eT[h] = e
                return eT

            def attn(b, eT, Vp):
                """attn@V + denominators (col-paired), normalize -> attnT."""
                attnT = []
                for c in range(NPAIR):
                    h0, h1 = 2 * c, 2 * c + 1
                    ap = apsum.tile([P, L], f32, name=f"ap_{c}_{b}", tag="ap")
                    dn = apsum.tile([P, L], f32, name=f"dn_{c}_{b}", tag="ap")
                    for j in range(NT):
                        qsl = slice(J_OFF[j], J_OFF[j] + J_NQ[j])
                        osl = slice(j * P, L)
                        st, sp_ = (j == 0), (j == NT - 1)
                        nc.tensor.matmul(
                            ap[0:HD, osl], Vp[j][:, c * P : c * P + HD],
                            eT[h0][:, qsl], start=st, stop=sp_,
                        )
                        nc.tensor.matmul(
                            ap[HD:P, osl], Vp[j][:, c * P + HD : (c + 1) * P],
                            eT[h1][:, qsl], start=st, stop=sp_,
                        )
                        nc.tensor.matmul(
                            dn[0:HD, osl], onesp[:, 0:HD],
                            eT[h0][:, qsl], start=st, stop=sp_,
                        )
                        nc.tensor.matmul(
                            dn[HD:P, osl], onesp[:, HD:P],
                            eT[h1][:, qsl], start=st, stop=sp_,
                        )
                    rc = ssp.tile([P, L], f32, name=f"rc_{c}_{b}", tag="rc")
                    nc.vector.reciprocal(rc, dn)
                    at = attp.tile([P, L], bf16, name=f"attnT_{c}_{b}", tag=f"attnT_{c}")
                    nc.vector.tensor_mul(at, ap, rc)
                    attnT.append(at)
                return attnT

            def out_proj(b, attnT):
                ysb = youtp.tile([P, NT, H], bf16, name=f"ysb_{b}", tag="y")
                for t in range(NT):
                    yp = ppsum.tile([P, H], f32, name=f"yp_{t}_{b}", tag="pp")
                    for kc in range(KC):
                        nc.tensor.matmul(
                            yp,
                            attnT[kc][:, t * P : (t + 1) * P],
                            Wo[:, kc, :],
                            start=(kc == 0),
                            stop=(kc == KC - 1),
                        )
                    cpy(ysb[:, t, :], yp)
                nc.sync.dma_start(out=out[b], in_=ysb)

            def body():
                # prologue: batch 0 inputs + projections
                xt, rels = dma_in(0)
                proj = project(0, xt)
                nxt = dma_in(1) if BL > 1 else None
                eT = None
                for b in range(BL):
                    # prefetch inputs for b+2
                    if b + 2 < BL:
                        pre = dma_in(b + 2)
                    eT_b = scores(b, proj[0], proj[1], proj[2], proj[3], rels)
                    Vp_b = proj[4]
                    # projections for b+1 run while DVE/ACT chew on scores(b)
                    if b + 1 < BL:
                        xt, rels = nxt
                        proj = project(b + 1, xt)
                        nxt = pre if b + 2 < BL else None
                    attnT = attn(b, eT_b, Vp_b)
                    out_proj(b, attnT)

            if iters > 1:
                with tc.For_i(0, iters, 1):
                    body()
            else:
                body()

    nc.compile()
    return nc


def _get_nc():
    global _built_nc
    if _built_nc is None:
        _built_nc = build_nc()
    return _built_nc


def make_host_inputs(inputs):
    """Full (unsharded) device-ready arrays: transposes, bf16/fp8 casts, SCALE
    folded into the Q-stream weights, causal mask folded into relative_time,
    rel packed into the per-k-tile q-concat layout."""
    import ml_dtypes

    bf = ml_dtypes.bfloat16
    f8 = ml_dtypes.float8_e4m3
    host = {}

    # x: [B, P, stream, kc, L]; stream order (brand, id, cate, v)
    xs = [
        np.asarray(inputs[n], dtype=np.float32)
        for n in ("side_brand", "seq_id", "side_cate", "V_id_input")
    ]
    xstk = np.stack(xs, axis=1)  # [B, s, L, H]
    xstk = xstk.reshape(B, 4, L, KC, P).transpose(0, 4, 1, 3, 2)  # [B, P, s, kc, L]
    host["xT"] = np.ascontiguousarray(xstk).astype(bf)

    # rel: transpose to [k, q], fold causal mask (additive MASK_VAL), fp8,
    # pack into [B, pair, P, hh, QTOT] with the (j, q >= j*P) concat layout
    rel = np.asarray(inputs["relative_time"], dtype=np.float32)
    mask = np.asarray(inputs["attn_mask"])  # [B, L, L] bool, True = keep
    rel = np.where(mask[:, None], rel, np.float32(MASK_VAL))
    rel = rel.transpose(0, 1, 3, 2)  # [B, NH, k, q]
    relp = np.empty((B, NPAIR, 2, P, QTOT), np.float32)
    for j in range(NT):
        seg = rel[:, :, j * P : (j + 1) * P, j * P :]  # [B, NH, P, nq]
        relp[..., J_OFF[j] : J_OFF[j] + J_NQ[j]] = seg.reshape(
            B, NPAIR, 2, P, J_NQ[j]
        )
    host["relc"] = np.ascontiguousarray(relp.transpose(0, 1, 3, 2, 4)).astype(f8)

    Wq = {
        s: np.asarray(inputs[f"Wq_{s}"], np.float32) * np.float32(SCALE)
        for s in ("id", "cate", "brand")
    }
    Wk = {s: np.asarray(inputs[f"Wk_{s}"], np.float32) for s in ("id", "cate", "brand")}

    # id+cate concat weights: [P, kc, h, s2, d] = W[h*64+d, kc*128+p]
    def icw(w_id, w_cate):
        w = np.stack([w_id, w_cate], axis=0)  # [2, H, H]
        w = w.reshape(2, NH, HD, KC, P)  # [s2, h, d, kc, p]
        return np.ascontiguousarray(w.transpose(4, 3, 1, 0, 2)).astype(bf)

    host["w_icq"] = icw(Wq["id"], Wq["cate"])
    host["w_ick"] = icw(Wk["id"], Wk["cate"])

    # simple transposed weights: [P, kc, g] = W[g, kc*128+p]
    def tw(w):
        return np.ascontiguousarray(w.reshape(H, KC, P).transpose(2, 1, 0)).astype(bf)

    host["w_bq"] = tw(Wq["brand"])
    host["w_bk"] = tw(Wk["brand"])
    host["w_v"] = tw(np.asarray(inputs["Wv"], np.float32))
    host["w_o"] = tw(np.asarray(inputs["Wo"], np.float32))
    host["identt"] = np.eye(P, dtype=np.float32).astype(f8)
    return host


def make_in_maps(inputs):
    host = make_host_inputs(inputs)
    in_maps = []
    for ci in range(NCORES):
        sl = slice(ci * BL, (ci + 1) * BL)
        m = {
            "xT": np.ascontiguousarray(host["xT"][sl]),
            "relc": np.ascontiguousarray(host["relc"][sl]),
        }
        for n in ("w_icq", "w_ick", "w_bq", "w_bk", "w_v", "w_o", "identt"):
            m[n] = host[n]
        in_maps.append(m)
    return in_maps


def run_sharded(inputs, trace=False):
    from concourse.bass_utils import run_bass_kernel_spmd

    nc = _get_nc()
    in_maps = make_in_maps(inputs)
    res = run_bass_kernel_spmd(nc, in_maps, core_ids=list(range(NCORES)), trace=trace)
    # out is [BL, P, NT, H] bf16: element (b,p,t,h) = y[b, t*128+p, h]
    outs = []
    for i in range(NCORES):
        y = np.asarray(res.results[i]["out"], dtype=np.float32)  # [BL, P, NT, H]
        outs.append(y.transpose(0, 2, 1, 3).reshape(BL, L, H))
    return np.concatenate(outs, axis=0), res


def kernel(**inputs) -> np.ndarray:
    y, _ = run_sharded(inputs, trace=False)
    return y


# revision 7
# speedup vs baseline: 1.1844x; 1.1844x over previous
"""Trainium2 Bass kernel for the DIFSR 3-stream attention block (v7).

Reference math (B=32, L=512, H=512, NH=8, HD=64):
    V     = heads(V_id_input @ Wv.T)                        # biases are all zero
    total = sum_s heads(x_s @ Wq_s.T) @ heads(x_s @ Wk_s.T).T * HD**-0.5
            for s in (id, cate, brand)
    total += relative_time;  causal mask;  softmax over k
    out   = (softmax @ V).merge_heads() @ Wo.T

Sharding: pure data-parallel over batch B across the 8 NeuronCores.

v5 (vs v4): exploits measured HW tile-position concurrency (col/row-paired
matmuls overlap on the PE array) and software-pipelines across batches so the
PE never waits on the DVE/ACT softmax chain.

  - id+cate Q/K projected into per-head concat tiles [64 id | 64 cate, L] via
    col-paired M=64 matmuls (concurrent on HW); scores for both streams come
    from ONE K=128 matmul per head. Brand scores: K=64 row-paired pairs.
  - Scores are computed TRANSPOSED sT[k, q] with causal block skipping; the
    per-(pair,head) masked rel bias is pre-packed on the host into the same
    q-concat layout [128, 1280] (fp8, additive -240 mask), so adds are 3 DVE
    ops and exp is ONE ACT op [128, 1280] per (pair, head).
  - attn@V redesigned: per-pair V tiles [V_h0 | V_h1] with col-paired matmuls
    (eT_h0 / eT_h1 as the two streams) accumulate attnT for BOTH heads into
    one psum tile; a ones-pair tile accumulates both denominators likewise.
    Normalization is ONE full-width reciprocal + ONE multiply per pair.
  - Emission order pipelines batches: scores(b) -> projections(b+1) ->
    attn(b) -> out-proj(b), so DVE/ACT softmax work for batch b runs under
    the projection matmuls of batch b+1.
  - Startup: brand weights + brand x stream DMA'd first so the PE starts
    ~3us in; remaining weights stream in under compute.
"""

import sys

if "/opt/trn_rl_repo" not in sys.path:
    sys.path.insert(0, "/opt/trn_rl_repo")

import numpy as np

B, L, H, NH = 32, 512, 512, 8
HD = H // NH  # 64
NCORES = 8
BL = B // NCORES  # 4 batches per core
SCALE = HD**-0.5
P = 128
NT = L // P  # 4 k/q tiles
KC = H // P  # 4 contraction chunks
NPAIR = NH // 2  # 4 head pairs
MASK_VAL = -240.0  # representable in fp8_e4m3; exp(s-240) == 0 in f32

# causal q-concat layout: for k-tile j, q ranges over [j*P, L)
J_OFF = [0, 512, 896, 1152]
J_NQ = [512, 384, 256, 128]
QTOT = 1280

_built_nc = None


def build_nc(iters=1):
    import concourse.mybir as mybir
    from concourse import bacc
    from concourse.tile import TileContext

    f32 = mybir.dt.float32
    bf16 = mybir.dt.bfloat16
    fp8 = mybir.dt.float8e4
    Exp = mybir.ActivationFunctionType.Exp

    nc = bacc.Bacc("TRN2", target_bir_lowering=False, debug=False)

    # host-prearranged inputs (see make_host_inputs for layouts)
    # xT stream order: 0=brand, 1=id, 2=cate, 3=v
    xT = nc.dram_tensor("xT", [BL, P, 4, KC, L], bf16, kind="ExternalInput").ap()
    relc = nc.dram_tensor("relc", [BL, NPAIR, P, 2, QTOT], fp8, kind="ExternalInput").ap()
    w_icq = nc.dram_tensor("w_icq", [P, KC, 2, NT, 2, HD], bf16, kind="ExternalInput").ap()
    w_ick = nc.dram_tensor("w_ick", [P, KC, 2, NT, 2, HD], bf16, kind="ExternalInput").ap()
    w_bq = nc.dram_tensor("w_bq", [P, KC, H], bf16, kind="ExternalInput").ap()
    w_bk = nc.dram_tensor("w_bk", [P, KC, H], bf16, kind="ExternalInput").ap()
    w_v = nc.dram_tensor("w_v", [P, KC, H], bf16, kind="ExternalInput").ap()
    w_o = nc.dram_tensor("w_o", [P, KC, H], bf16, kind="ExternalInput").ap()
    out = nc.dram_tensor("out", [BL, P, NT, H], bf16, kind="ExternalOutput").ap()

    with TileContext(nc) as tc:
        with (
            tc.tile_pool(name="wsb", bufs=1) as wsb,
            tc.tile_pool(name="xt", bufs=2) as xtp,
            tc.tile_pool(name="rel", bufs=2) as relp,
            tc.tile_pool(name="qk", bufs=2) as qkp,
            tc.tile_pool(name="vp", bufs=2) as vpp,
            tc.tile_pool(name="ssb", bufs=2) as ssp,
            tc.tile_pool(name="et", bufs=2) as etp,
            tc.tile_pool(name="att", bufs=2) as attp,
            tc.tile_pool(name="yout", bufs=1) as youtp,
            tc.tile_pool(name="ppsum", bufs=3, space="PSUM") as ppsum,
            tc.tile_pool(name="spsum", bufs=3, space="PSUM") as spsum,
            tc.tile_pool(name="apsum", bufs=2, space="PSUM") as apsum,
        ):
            # ---- weights, brand first so the PE can start early ----
            Wbq = wsb.tile([P, KC, H], bf16, name="Wbq")
            nc.sync.dma_start(out=Wbq, in_=w_bq)
            Wbk = wsb.tile([P, KC, H], bf16, name="Wbk")
            nc.sync.dma_start(out=Wbk, in_=w_bk)

            xt0 = xtp.tile([P, 4, KC, L], bf16, name="xt_0", tag="xt")
            nc.sync.dma_start(out=xt0[:, 0], in_=xT[0, :, 0])  # brand stream

            Wicq = wsb.tile([P, KC, 2, NT, 2, HD], bf16, name="Wicq")
            nc.sync.dma_start(out=Wicq, in_=w_icq)
            Wick = wsb.tile([P, KC, 2, NT, 2, HD], bf16, name="Wick")
            nc.sync.dma_start(out=Wick, in_=w_ick)
            nc.sync.dma_start(out=xt0[:, 1:3], in_=xT[0, :, 1:3])  # id+cate
            nc.sync.dma_start(out=xt0[:, 3], in_=xT[0, :, 3])  # v
            Wv = wsb.tile([P, KC, H], bf16, name="Wv")
            nc.sync.dma_start(out=Wv, in_=w_v)
            Wo = wsb.tile([P, KC, H], bf16, name="Wo")
            nc.sync.dma_start(out=Wo, in_=w_o)

            # ones pair tile for the denominator matmuls
            onesp = wsb.tile([P, P], bf16, name="onesp")
            nc.gpsimd.memset(onesp, 1.0)

            # PSUM->SBUF drains alternate engines; DVE carries adds/recip/mul
            rr = [0]

            def cpy(dst, src):
                rr[0] += 1
                if rr[0] % 3 == 0:
                    nc.vector.tensor_copy(dst, src)
                else:
                    nc.scalar.copy(dst, src)

            def dma_in(b):
                """Prefetch xt(b) and rel(b); returns (xt, rels)."""
                if b == 0:
                    xt = xt0
                else:
                    xt = xtp.tile([P, 4, KC, L], bf16, name=f"xt_{b}", tag="xt")
                    nc.sync.dma_start(out=xt, in_=xT[b])
                rels = []
                for c in range(NPAIR):
                    rl = relp.tile([P, 2, QTOT], fp8, name=f"rel_{c}_{b}", tag=f"rel{c}")
                    nc.sync.dma_start(out=rl, in_=relc[b, c])
                    rels.append(rl)
                return xt, rels

            def project(b, xt):
                """All projections for batch b -> (Qic, Kic, Qb, Kb, Vp)."""
                # brand Q/K: head-pair chunk tiles [h2c | h2c+1, L]
                def project_b(wsb_t, kind):
                    tiles = []
                    for c in range(NPAIR):
                        pp = ppsum.tile([P, L], f32, name=f"pp{kind}_{c}_{b}", tag="pp")
                        for kc in range(KC):
                            nc.tensor.matmul(
                                pp,
                                wsb_t[:, kc, c * P : (c + 1) * P],
                                xt[:, 0, kc, :],
                                start=(kc == 0),
                                stop=(kc == KC - 1),
                            )
                        t = qkp.tile([P, L], bf16, name=f"{kind}_{c}_{b}", tag=f"{kind}_{c}")
                        cpy(t, pp)
                        tiles.append(t)
                    return tiles

                Qb = project_b(Wbq, "Qb")
                Kb = project_b(Wbk, "Kb")

                # id+cate Q/K: full-array M=128 matmuls, one tile per
                # (stream, head-pair). Even heads get concat tiles [id | cate],
                # odd heads [cate | id] (cate weight tiles are head-swapped on
                # the host), so every drain half-copy keeps its partition range.
                def project_ic(wsb_t, kind):
                    tiles = [
                        qkp.tile([P, L], bf16, name=f"{kind}_{h}_{b}", tag=f"{kind}_{h}")
                        for h in range(NH)
                    ]
                    for s2 in range(2):  # 0=id, 1=cate
                        for tp in range(NT):
                            pp = ppsum.tile(
                                [P, L], f32, name=f"pp{kind}_{s2}_{tp}_{b}", tag="pp"
                            )
                            for kc in range(KC):
                                nc.tensor.matmul(
                                    pp,
                                    wsb_t[:, kc, s2, tp],
                                    xt[:, 1 + s2, kc, :],
                                    start=(kc == 0),
                                    stop=(kc == KC - 1),
                                )
                            if s2 == 0:  # id tile: heads (2tp, 2tp+1)
                                cpy(tiles[2 * tp][0:HD, :], pp[0:HD, :])
                                cpy(tiles[2 * tp + 1][HD:P, :], pp[HD:P, :])
                            else:  # cate tile: heads (2tp+1, 2tp)
                                cpy(tiles[2 * tp + 1][0:HD, :], pp[0:HD, :])
                                cpy(tiles[2 * tp][HD:P, :], pp[HD:P, :])
                    return tiles

                Qic = project_ic(Wicq, "Qic")
                Kic = project_ic(Wick, "Kic")

                # V into per-pair tiles: Vp[j][:, c*128:(c+1)*128] = [V_h0 | V_h1]
                Vp = []
                for j in range(NT):
                    pp = ppsum.tile([P, H], f32, name=f"ppv_{j}_{b}", tag="pp")
                    for kc in range(KC):
                        nc.tensor.matmul(
                            pp,
                            xt[:, 3, kc, j * P : (j + 1) * P],
                            Wv[:, kc, :],
                            start=(kc == 0),
                            stop=(kc == KC - 1),
                        )
                    t = vpp.tile([P, H], bf16, name=f"Vp_{j}_{b}", tag=f"Vp_{j}")
                    cpy(t, pp)
                    Vp.append(t)
                return Qic, Kic, Qb, Kb, Vp

            def scores(b, Qic, Kic, Qb, Kb, rels):
                """Masked scores -> exp, as eT concat tiles [128, QTOT] bf16."""
                eT = {}
                for c in range(NPAIR):
                    rl = rels[c]
                    for hh in range(2):
                        h = 2 * c + hh
                        hsl = slice(hh * HD, (hh + 1) * HD)
                        sp = {}
                        for grp, js in (("j0", (0,)), ("j1", (1,)), ("j23", (2, 3))):
                            w = sum(J_NQ[j] for j in js)
                            t = spsum.tile([P, w], f32, name=f"sp{grp}_{h}_{b}", tag="sp")
                            off = 0
                            for j in js:
                                nq = J_NQ[j]
                                jsl = slice(j * P, (j + 1) * P)
                                qsl = slice(j * P, L)
                                nc.tensor.matmul(
                                    t[:, off : off + nq],
                                    Kic[h][:, jsl],
                                    Qic[h][:, qsl],
                                    start=True,
                                    stop=False,
                                )
                                nc.tensor.matmul(
                                    t[:, off : off + nq],
                                    Kb[c][hsl, jsl],
                                    Qb[c][hsl, qsl],
                                    start=False,
                                    stop=True,
                                )
                                off += nq
                            sp[grp] = t
                        ss = ssp.tile([P, QTOT], f32, name=f"ss_{h}_{b}", tag="ss")
                        nc.vector.tensor_add(
                            ss[:, 0:512], sp["j0"], rl[:, hh, 0:512]
                        )
                        nc.vector.tensor_add(
                            ss[:, 512:896], sp["j1"], rl[:, hh, 512:896]
                        )
                        nc.vector.tensor_add(
                            ss[:, 896:1280], sp["j23"], rl[:, hh, 896:1280]
                        )
                        e = etp.tile([P, QTOT], bf16, name=f"eT_{h}_{b}", tag=f"eT_{h}")
                        nc.scalar.activation(e, ss, Exp)
                        eT[h] = e
                return eT

            def attn(b, eT, Vp):
                """attn@V + denominators (col-paired), normalize -> attnT."""
                attnT = []
                for c in range(NPAIR):
                    h0, h1 = 2 * c, 2 * c + 1
                    ap = apsum.tile([P, L], f32, name=f"ap_{c}_{b}", tag="ap")
                    dn = apsum.tile([P, L], f32, name=f"dn_{c}_{b}", tag="ap")
                    for j in range(NT):
                        qsl = slice(J_OFF[j], J_OFF[j] + J_NQ[j])
                        osl = slice(j * P, L)
                        st, sp_ = (j == 0), (j == NT - 1)
                        nc.tensor.matmul(
                            ap[0:HD, osl], Vp[j][:, c * P : c * P + HD],
                            eT[h0][:, qsl], start=st, stop=sp_,
                        )
                        nc.tensor.matmul(
                            ap[HD:P, osl], Vp[j][:, c * P + HD : (c + 1) * P],
                            eT[h1][:, qsl], start=st, stop=sp_,
                        )
                        nc.tensor.matmul(
                            dn[0:HD, osl], onesp[:, 0:HD],
                            eT[h0][:, qsl], start=st, stop=sp_,
                        )
                        nc.tensor.matmul(
                            dn[HD:P, osl], onesp[:, HD:P],
                            eT[h1][:, qsl], start=st, stop=sp_,
                        )
                    rc = ssp.tile([P, L], f32, name=f"rc_{c}_{b}", tag="rc")
                    nc.vector.reciprocal(rc, dn)
                    at = attp.tile([P, L], bf16, name=f"attnT_{c}_{b}", tag=f"attnT_{c}")
                    nc.vector.tensor_mul(at, ap, rc)
                    attnT.append(at)
                return attnT

            def out_proj(b, attnT):
                ysb = youtp.tile([P, NT, H], bf16, name=f"ysb_{b}", tag="y")
                for t in range(NT):
                    yp = ppsum.tile([P, H], f32, name=f"yp_{t}_{b}", tag="pp")
                    for kc in range(KC):
                        nc.tensor.matmul(
                            yp,
                            attnT[kc][:, t * P : (t + 1) * P],
                            Wo[:, kc, :],
                            start=(kc == 0),
                            stop=(kc == KC - 1),
                        )
                    cpy(ysb[:, t, :], yp)
                nc.sync.dma_start(out=out[b], in_=ysb)

            def body():
                # prologue: batch 0 inputs + projections
                xt, rels = dma_in(0)
                proj = project(0, xt)
                nxt = dma_in(1) if BL > 1 else None
                eT = None
                for b in range(BL):
                    # prefetch inputs for b+2
                    if b + 2 < BL:
                        pre = dma_in(b + 2)
                    eT_b = scores(b, proj[0], proj[1], proj[2], proj[3], rels)
                    Vp_b = proj[4]
                    # projections for b+1 run while DVE/ACT chew on scores(b)
                    if b + 1 < BL:
                        xt, rels = nxt
                        proj = project(b + 1, xt)
                        nxt = pre if b + 2 < BL else None
                    attnT = attn(b, eT_b, Vp_b)
                    out_proj(b, attnT)

            if iters > 1:
                with tc.For_i(0, iters, 1):
                    body()
            else:
                body()

    nc.compile()
    return nc


def _get_nc():
    global _built_nc
    if _built_nc is None:
        _built_nc = build_nc()
    return _built_nc


def make_host_inputs(inputs):
    """Full (unsharded) device-ready arrays: transposes, bf16/fp8 casts, SCALE
    folded into the Q-stream weights, causal mask folded into relative_time,
    rel packed into the per-k-tile q-concat layout."""
    import ml_dtypes

    bf = ml_dtypes.bfloat16
    f8 = ml_dtypes.float8_e4m3
    host = {}

    # x: [B, P, stream, kc, L]; stream order (brand, id, cate, v)
    xs = [
        np.asarray(inputs[n], dtype=np.float32)
        for n in ("side_brand", "seq_id", "side_cate", "V_id_input")
    ]
    xstk = np.stack(xs, axis=1)  # [B, s, L, H]
    xstk = xstk.reshape(B, 4, L, KC, P).transpose(0, 4, 1, 3, 2)  # [B, P, s, kc, L]
    host["xT"] = np.ascontiguousarray(xstk).astype(bf)

    # rel: transpose to [k, q], fold causal mask (additive MASK_VAL), fp8,
    # pack into [B, pair, P, hh, QTOT] with the (j, q >= j*P) concat layout
    rel = np.asarray(inputs["relative_time"], dtype=np.float32)
    mask = np.asarray(inputs["attn_mask"])  # [B, L, L] bool, True = keep
    rel = np.where(mask[:, None], rel, np.float32(MASK_VAL))
    rel = rel.transpose(0, 1, 3, 2)  # [B, NH, k, q]
    relp = np.empty((B, NPAIR, 2, P, QTOT), np.float32)
    for j in range(NT):
        seg = rel[:, :, j * P : (j + 1) * P, j * P :]  # [B, NH, P, nq]
        relp[..., J_OFF[j] : J_OFF[j] + J_NQ[j]] = seg.reshape(
            B, NPAIR, 2, P, J_NQ[j]
        )
    host["relc"] = np.ascontiguousarray(relp.transpose(0, 1, 3, 2, 4)).astype(f8)

    Wq = {
        s: np.asarray(inputs[f"Wq_{s}"], np.float32) * np.float32(SCALE)
        for s in ("id", "cate", "brand")
    }
    Wk = {s: np.asarray(inputs[f"Wk_{s}"], np.float32) for s in ("id", "cate", "brand")}

    # id+cate weights: [P, kc, s2, tile, pairidx, d]; id tile t holds heads
    # (2t, 2t+1), cate tile t holds heads (2t+1, 2t) (swapped - see kernel)
    def icw(w_id, w_cate):
        w = np.stack([w_id, w_cate], axis=0)  # [2, H, H]
        w = w.reshape(2, NT, 2, HD, KC, P)  # [s2, tile, pairidx, d, kc, p]
        w = np.concatenate(
            [w[0:1], w[1:2, :, ::-1]], axis=0
        )  # swap head order within cate tiles
        return np.ascontiguousarray(w.transpose(5, 4, 0, 1, 2, 3)).astype(bf)

    host["w_icq"] = icw(Wq["id"], Wq["cate"])
    host["w_ick"] = icw(Wk["id"], Wk["cate"])

    # simple transposed weights: [P, kc, g] = W[g, kc*128+p]
    def tw(w):
        return np.ascontiguousarray(w.reshape(H, KC, P).transpose(2, 1, 0)).astype(bf)

    host["w_bq"] = tw(Wq["brand"])
    host["w_bk"] = tw(Wk["brand"])
    host["w_v"] = tw(np.asarray(inputs["Wv"], np.float32))
    host["w_o"] = tw(np.asarray(inputs["Wo"], np.float32))
    return host


def make_in_maps(inputs):
    host = make_host_inputs(inputs)
    in_maps = []
    for ci in range(NCORES):
        sl = slice(ci * BL, (ci + 1) * BL)
        m = {
            "xT": np.ascontiguousarray(host["xT"][sl]),
            "relc": np.ascontiguousarray(host["relc"][sl]),
        }
        for n in ("w_icq", "w_ick", "w_bq", "w_bk", "w_v", "w_o"):
            m[n] = host[n]
        in_maps.append(m)
    return in_maps


def run_sharded(inputs, trace=False):
    from concourse.bass_utils import run_bass_kernel_spmd

    nc = _get_nc()
    in_maps = make_in_maps(inputs)
    res = run_bass_kernel_spmd(nc, in_maps, core_ids=list(range(NCORES)), trace=trace)
    # out is [BL, P, NT, H] bf16: element (b,p,t,h) = y[b, t*128+p, h]
    outs = []
    for i in range(NCORES):
        y = np.asarray(res.results[i]["out"], dtype=np.float32)  # [BL, P, NT, H]
        outs.append(y.transpose(0, 2, 1, 3).reshape(BL, L, H))
    return np.concatenate(outs, axis=0), res


def kernel(**inputs) -> np.ndarray:
    y, _ = run_sharded(inputs, trace=False)
    return y
